# revision 17
# baseline (speedup 1.0000x reference)
"""Trainium2 Bass kernel for nn_Attention_Module (SAGAN-style attention block).

Reference computation (per batch item b):
    f  = maxpool2(relu(bn1(conv1x1_1(x))))   # (C/8, H/2*W/2) = (32, 1024)
    g  = relu(bn2(conv1x1_2(x)))             # (C/8, H*W)     = (32, 4096)
    hh = maxpool2(relu(bn3(conv1x1_3(x))))   # (C/2, 1024)    = (128, 1024)
    s[n, m] = sum_k f[k, n] * g[k, m]        # (1024, 4096)
    beta = softmax(s, axis=n)
    o  = hh @ beta                           # (128, 4096)
    out = gamma * bn4(conv1x1_4(o)) + x

Sharding: data-parallel over batch B=8 across the 8 NeuronCores (one item per
core), SPMD NEFF with per-core input maps. No collectives.

Kernel layout choices:
  - conv+BN folded host-side into (scaled weight, bias); convs are matmuls
    with channels on the partition dim.  bias+relu on ScalarE straight out of
    PSUM; relu commutes with maxpool so the 2x2 pool runs after on VectorE.
  - f and g are materialized 4x-replicated across partition groups so the
    s-matmul (contraction K=32) can run as 4 concurrent row-tiles
    (tile_position=(32i, 0)) at ~4x throughput.
  - s is produced with n on partitions, m on free dim.  softmax over n
    (partition axis) is done as: E=exp(s) (ScalarE), column sums via a
    matmul with an all-ones stationary operand (which also broadcasts the
    sum to all 128 partitions), and the division is applied to the 128-row
    o matrix instead of the 1024-row beta matrix (conv4 commutes with the
    per-column scale).
  - all matmuls stream as float32r (full PE rate at N>=256).
"""

import sys

sys.path.insert(0, "/opt/trn_rl_repo")

import numpy as np

import concourse.bass as bass  # noqa: F401  (re-exported for tooling)
import concourse.tile as tile
from concourse import bacc, mybir
from concourse.bass import ts

F32 = mybir.dt.float32
F32R = mybir.dt.float32r
F8 = mybir.dt.float8e4
DR = mybir.MatmulPerfMode.DoubleRow

P = 128          # SBUF partitions
C = 256          # input channels
C8 = 32          # conv1/conv2 output channels
C2 = 128         # conv3 output channels
H = W = 64
HW = H * W       # 4096
HW4 = HW // 4    # 1024 (pooled spatial)
MB = 512         # m-block (free-dim tile)
NB = HW // MB    # 8 m-blocks
NCH = HW4 // P   # 8 n-chunks of 128
EPS = 1e-5
N_CORES = 8

AOP = mybir.AluOpType


def build_nc(reps: int = 1):
    nc = bacc.Bacc(
        "TRN2", target_bir_lowering=False, debug=False, num_devices=N_CORES
    )

    x_d = nc.dram_tensor("x", [2, P, HW], F32R, kind="ExternalInput")
    w1_d = nc.dram_tensor("w1x4", [2, P, P], F32R, kind="ExternalInput")
    w2_d = nc.dram_tensor("w2x4", [2, P, P], F32R, kind="ExternalInput")
    w3_d = nc.dram_tensor("w3t", [2, P, P], F32R, kind="ExternalInput")
    w4_d = nc.dram_tensor("w4t", [P, C], F32R, kind="ExternalInput")
    c1_d = nc.dram_tensor("c1x4", [P, 1], F32, kind="ExternalInput")
    c2_d = nc.dram_tensor("c2x4", [P, 1], F32, kind="ExternalInput")
    c3_d = nc.dram_tensor("c3", [P, 1], F32, kind="ExternalInput")
    c4_d = nc.dram_tensor("c4", [P, 2], F32, kind="ExternalInput")
    id_d = nc.dram_tensor("ident", [P, P], F32, kind="ExternalInput")
    ones_d = nc.dram_tensor("ones", [P, 2, P], F8, kind="ExternalInput")
    out_d = nc.dram_tensor("out", [2, P, HW], F32, kind="ExternalOutput")

    with tile.TileContext(nc) as tc:
        with (
            tc.tile_pool(name="const", bufs=1) as const,
            tc.tile_pool(name="big", bufs=1) as big,
            tc.tile_pool(name="tmpb", bufs=4) as tmpb,
            tc.tile_pool(name="epool", bufs=6) as epool,
            tc.tile_pool(name="osb", bufs=2) as osb_pool,
            tc.tile_pool(name="rsb", bufs=2) as rsb_pool,
            tc.tile_pool(name="outsb", bufs=4) as outsb_pool,
        ):
            # ---- parameter loads ----
            w1_sb = const.tile([P, 2, P], F32R)
            w2_sb = const.tile([P, 2, P], F32R)
            w3_sb = const.tile([P, 2, P], F32R)
            w4_sb = const.tile([P, 2, P], F32R)
            for k in range(2):
                nc.sync.dma_start(out=w1_sb[:, k, :], in_=w1_d[k])
                nc.sync.dma_start(out=w2_sb[:, k, :], in_=w2_d[k])
                nc.sync.dma_start(out=w3_sb[:, k, :], in_=w3_d[k])
                nc.sync.dma_start(out=w4_sb[:, k, :], in_=w4_d[:, ts(k, P)])
            c1_sb = const.tile([P, 1], F32)
            c2_sb = const.tile([P, 1], F32)
            c3_sb = const.tile([P, 1], F32)
            c4_sb = const.tile([P, 2], F32)
            nc.sync.dma_start(out=c1_sb, in_=c1_d[:, :])
            nc.sync.dma_start(out=c2_sb, in_=c2_d[:, :])
            nc.sync.dma_start(out=c3_sb, in_=c3_d[:, :])
            nc.sync.dma_start(out=c4_sb, in_=c4_d[:, :])
            ident_sb = const.tile([P, P], F32)
            nc.sync.dma_start(out=ident_sb, in_=id_d[:, :])
            ones_sb = const.tile([P, 2, P], F8)
            nc.sync.dma_start(out=ones_sb, in_=ones_d[:, :, :])

            # ---- input load (quartered for load/compute overlap) ----
            x_sb = [
                big.tile([P, HW], F32R, tag=f"x{c}", name=f"x_sb{c}")
                for c in range(2)
            ]
            for q in range(NB):
                for c in range(2):
                    nc.sync.dma_start(
                        out=x_sb[c][:, ts(q, MB)],
                        in_=x_d[c, :, ts(q, MB)],
                    )

            F4 = big.tile([P, HW4], F32R, tag="F4")
            G4 = big.tile([P, HW], F32R, tag="G4")
            hh = big.tile([P, HW4], F32, tag="hh")
            hhT = big.tile([P, NCH, P], F8, tag="hhT")

            def conv_mm(ps, w_sb, t):
                for c in range(2):
                    nc.tensor.matmul(
                        ps,
                        lhsT=w_sb[:, c, :],
                        rhs=x_sb[c][:, ts(t, MB)],
                        start=(c == 0),
                        stop=(c == 1),
                    )

            def pool_bias_relu(ps, dest_128, c_sb):
                # ps: (128, 512) conv psum covering 8 rows x 64 cols.
                # relu(x + bias) commutes with maxpool, so ACT does bias+relu
                # from PSUM, then DVE does the 2x2 maxpool SBUF->SBUF.
                y = tmpb.tile([P, MB], F32, tag="y")
                nc.scalar.activation(
                    out=y,
                    in_=ps,
                    func=mybir.ActivationFunctionType.Relu,
                    bias=c_sb,
                )
                yv = y.rearrange("p (h e w d) -> p h e w d", h=4, e=2, w=32, d=2)
                t1 = tmpb.tile([P, 4, 2, 32], F32, tag="t1")
                nc.vector.tensor_max(t1, yv[:, :, :, :, 0], yv[:, :, :, :, 1])
                nc.vector.tensor_max(
                    dest_128.rearrange("p (a b) -> p a b", a=4),
                    t1[:, :, 0, :],
                    t1[:, :, 1, :],
                )

            for _rep in range(reps):
                # ---- phase B: convs 1-3 + hh transpose ----
                with (
                    tc.tile_pool(name="pscv", bufs=6, space="PSUM") as pscv,
                    tc.tile_pool(name="pstr", bufs=2, space="PSUM") as pstr,
                ):
                    for t in range(NB):  # conv1 -> F4 (4x replicated f)
                        ps = pscv.tile([P, MB], F32, tag="cv")
                        conv_mm(ps, w1_sb, t)
                        pool_bias_relu(ps, F4[:, ts(t, P)], c1_sb)
                    for t in range(NB):  # conv2 -> G4 (4x replicated g)
                        ps = pscv.tile([P, MB], F32, tag="cv")
                        conv_mm(ps, w2_sb, t)
                        nc.vector.tensor_scalar(
                            out=G4[:, ts(t, MB)],
                            in0=ps,
                            scalar1=c2_sb,
                            scalar2=0.0,
                            op0=AOP.add,
                            op1=AOP.max,
                        )
                    for t in range(NB):  # conv3 -> hh
                        ps = pscv.tile([P, MB], F32, tag="cv")
                        conv_mm(ps, w3_sb, t)
                        pool_bias_relu(ps, hh[:, ts(t, P)], c3_sb)
                    for j in range(NCH):  # hh -> hhT (PE transpose)
                        tp = pstr.tile([P, P], F32, tag="tr")
                        nc.tensor.transpose(tp, hh[:, ts(j, P)], ident_sb)
                        nc.vector.tensor_copy(out=hhT[:, j, :], in_=tp)

                # ---- phase C: attention + conv4 + residual, per m-block ----
                with (
                    tc.tile_pool(name="pss", bufs=2, space="PSUM") as pss,
                    tc.tile_pool(name="pso", bufs=1, space="PSUM") as pso,
                    tc.tile_pool(name="psr", bufs=1, space="PSUM") as psr,
                    tc.tile_pool(name="psy", bufs=2, space="PSUM") as psy,
                ):

                    def conv4_residual(t, o_sb):
                        for h in range(2):
                            y_ps = psy.tile([P, MB], F32, tag="y")
                            nc.tensor.matmul(
                                y_ps,
                                lhsT=w4_sb[:, h, :],
                                rhs=o_sb,
                                start=True,
                                stop=True,
                            )
                            ob = outsb_pool.tile([P, MB], F32, tag="ob")
                            nc.vector.scalar_tensor_tensor(
                                out=ob,
                                in0=y_ps,
                                scalar=c4_sb[:, h : h + 1],
                                in1=x_sb[h][:, ts(t, MB)].bitcast(F32),
                                op0=AOP.add,
                                op1=AOP.add,
                            )
                            nc.sync.dma_start(out=out_d[h, :, ts(t, MB)], in_=ob)

                    prev = None  # (t, o_sb) of previous block, conv4 deferred
                    for t in range(NB):
                        # scores s[n, m] for n-chunks j=0..7, 4x row-packed
                        e_tiles = []
                        for g in range(2):
                            sps = [
                                pss.tile([P, 2, MB], F32, tag="s", name=f"sp{t}{g}0"),
                                pss.tile([P, 2, MB], F32, tag="s", name=f"sp{t}{g}1"),
                            ]
                            for i in range(4):
                                j = 4 * g + i
                                nc.tensor.matmul(
                                    sps[i // 2][:, i % 2, :],
                                    lhsT=F4[32 * i : 32 * (i + 1), ts(j, P)],
                                    rhs=G4[32 * i : 32 * (i + 1), ts(t, MB)],
                                    start=True,
                                    stop=True,
                                    tile_position=(32 * i, 0),
                                )
                            for sp in sps:
                                e = epool.tile([P, 2, MB], F8, tag="e")
                                nc.scalar.activation(
                                    out=e,
                                    in_=sp,
                                    func=mybir.ActivationFunctionType.Exp,
                                )
                                e_tiles.append(e)

                        # column sums of E (all-ones stationary), broadcast to
                        # all partitions; reciprocal overlaps the o-matmuls
                        r_ps = psr.tile([P, MB], F32, tag="r")
                        for q in range(NCH // 2):
                            nc.tensor.matmul(
                                r_ps,
                                lhsT=ones_sb,
                                rhs=e_tiles[q],
                                start=(q == 0),
                                stop=(q == NCH // 2 - 1),
                                perf_mode=DR,
                            )
                        r_sb = rsb_pool.tile([P, MB], F32, tag="r")
                        nc.vector.reciprocal(r_sb, r_ps)

                        # o = hh @ E (accumulate over n-chunks)
                        o_ps = pso.tile([P, MB], F32, tag="o")
                        for q in range(NCH // 2):
                            nc.tensor.matmul(
                                o_ps,
                                lhsT=hhT[:, 2 * q : 2 * q + 2, :],
                                rhs=e_tiles[q],
                                start=(q == 0),
                                stop=(q == NCH // 2 - 1),
                                perf_mode=DR,
                            )
                        o_sb = osb_pool.tile([P, MB], F32R, tag="o")
                        nc.vector.tensor_mul(o_sb, o_ps, r_sb)

                        # conv4 of the PREVIOUS block: keeps PE busy while
                        # DVE finishes this block's divide
                        if prev is not None:
                            conv4_residual(*prev)
                        prev = (t, o_sb)
                    conv4_residual(*prev)

    nc.compile()
    return nc


def _fold(w, b, s, t, m, v):
    w = np.asarray(w, np.float64)
    a = np.asarray(s, np.float64) / np.sqrt(np.asarray(v, np.float64) + EPS)
    W = w * a[:, None]
    c = (np.asarray(b, np.float64) - np.asarray(m, np.float64)) * a + np.asarray(
        t, np.float64
    )
    return W, c


def _np_f8():
    return mybir.dt.np(F8)


def make_in_maps(inputs):
    x = np.ascontiguousarray(np.asarray(inputs["x"], np.float32))  # (8,256,64,64)
    gamma = float(np.asarray(inputs["gamma"]))

    W1, c1 = _fold(*(inputs[f"{k}1"] for k in "wbstmv"))
    W2, c2 = _fold(*(inputs[f"{k}2"] for k in "wbstmv"))
    W3, c3 = _fold(*(inputs[f"{k}3"] for k in "wbstmv"))
    W4, c4 = _fold(*(inputs[f"{k}4"] for k in "wbstmv"))

    f32 = np.float32
    shared = {
        "w1x4": np.ascontiguousarray(
            np.tile(W1.T, (1, 4)).reshape(2, P, P).astype(f32)
        ),
        "w2x4": np.ascontiguousarray(
            np.tile(W2.T, (1, 4)).reshape(2, P, P).astype(f32)
        ),
        "w3t": np.ascontiguousarray(W3.T.reshape(2, P, P).astype(f32)),
        "w4t": np.ascontiguousarray((gamma * W4).T.astype(f32)),
        "c1x4": np.tile(c1, 4).reshape(P, 1).astype(f32),
        "c2x4": np.tile(c2, 4).reshape(P, 1).astype(f32),
        "c3": c3.reshape(P, 1).astype(f32),
        "c4": np.ascontiguousarray((gamma * c4).reshape(2, P).T.astype(f32)),
        "ident": np.eye(P, dtype=f32),
        "ones": np.ones((P, 2, P), _np_f8()),
    }
    return [
        {"x": np.ascontiguousarray(x[bb].reshape(2, P, HW)), **shared}
        for bb in range(x.shape[0])
    ]


_CACHE = {}


def _get_runner():
    """Build + compile the Bass module once, and return a cached callable
    that executes it on the 8 cores (jit-compiled once, reusable)."""
    if "runner" in _CACHE:
        return _CACHE["runner"]

    import jax
    from jax.sharding import Mesh, PartitionSpec
    from jax.experimental.shard_map import shard_map

    from concourse import bass2jax
    from concourse.bass2jax import _bass_exec_p, partition_id_tensor

    nc = build_nc()
    bass2jax.install_neuronx_cc_hook()

    partition_name = (
        nc.partition_id_tensor.name if nc.partition_id_tensor else None
    )
    in_names, out_names, out_avals, zero_outs = [], [], [], []
    for alloc in nc.m.functions[0].allocations:
        if not isinstance(alloc, mybir.MemoryLocationSet):
            continue
        name = alloc.memorylocations[0].name
        if alloc.kind == "ExternalInput":
            if name != partition_name:
                in_names.append(name)
        elif alloc.kind == "ExternalOutput":
            out_names.append(name)
            shape = tuple(alloc.tensor_shape)
            dtype = mybir.dt.np(alloc.dtype)
            out_avals.append(jax.core.ShapedArray(shape, dtype))
            zero_outs.append(np.zeros(shape, dtype))
    n_params = len(in_names)
    n_outs = len(out_avals)
    all_in_names = list(in_names) + list(out_names)
    if partition_name is not None:
        all_in_names = all_in_names + [partition_name]

    def _body(*args):
        operands = list(args)
        if partition_name is not None:
            operands.append(partition_id_tensor())
        outs = _bass_exec_p.bind(
            *operands,
            out_avals=tuple(out_avals),
            in_names=tuple(all_in_names),
            out_names=tuple(out_names),
            lowering_input_output_aliases=(),
            sim_require_finite=True,
            sim_require_nnan=True,
            nc=nc,
        )
        return tuple(outs)

    devices = jax.devices()[:N_CORES]
    mesh = Mesh(np.asarray(devices), ("core",))
    in_specs = (PartitionSpec("core"),) * (n_params + n_outs)
    out_specs = (PartitionSpec("core"),) * n_outs
    sharded = jax.jit(
        shard_map(
            _body, mesh=mesh, in_specs=in_specs, out_specs=out_specs, check_rep=False
        ),
        donate_argnums=tuple(range(n_params, n_params + n_outs)),
        keep_unused=True,
    )

    def run(in_maps):
        concat_in = [
            np.concatenate([np.asarray(m[name]) for m in in_maps], axis=0)
            for name in in_names
        ]
        concat_zeros = [
            np.zeros((N_CORES * z.shape[0], *z.shape[1:]), z.dtype)
            for z in zero_outs
        ]
        out_arrs = sharded(*concat_in, *concat_zeros)
        return [
            {
                name: np.asarray(out_arrs[i]).reshape(
                    N_CORES, *out_avals[i].shape
                )[cc]
                for i, name in enumerate(out_names)
            }
            for cc in range(N_CORES)
        ]

    _CACHE["runner"] = run
    return run


def kernel(**inputs) -> np.ndarray:
    run = _get_runner()
    in_maps = make_in_maps(inputs)
    results = run(in_maps)
    out = np.stack(
        [results[bb]["out"].reshape(C, H, W) for bb in range(N_CORES)]
    )
    return out.astype(np.float32)


if __name__ == "__main__":
    rng = np.random.default_rng(0)
    fake = {"x": rng.standard_normal((8, C, H, W), dtype=np.float32)}
    for i, (oc, ic) in zip([1, 2, 3, 4], [(C8, C), (C8, C), (C2, C), (C, C2)]):
        fake[f"w{i}"] = rng.standard_normal((oc, ic), dtype=np.float32) * 0.01
        fake[f"b{i}"] = np.zeros(oc, np.float32)
        fake[f"s{i}"] = rng.uniform(0.5, 1.5, oc).astype(np.float32)
        fake[f"t{i}"] = rng.standard_normal(oc).astype(np.float32) * 0.1
        fake[f"m{i}"] = rng.standard_normal(oc).astype(np.float32) * 0.1
        fake[f"v{i}"] = rng.uniform(0.5, 1.5, oc).astype(np.float32)
    fake["gamma"] = np.float32(0.5)
    out = kernel(**fake)
    print("out", out.shape, out.dtype, float(np.abs(out).mean()))
